# revision 4
# baseline (speedup 1.0000x reference)
"""Trainium2 Bass kernel for nn_Decoder_30983894073702.

Architecture: 8-way model parallel over HIDDEN=1024 (128 units/core),
batch=128 kept whole as the matmul moving (free) dimension, bf16 compute
with fp32 PSUM accumulation and fp32 c-state.

Per step: two AllGathers ({h0',c0'} after cell0, {h1',c1'} after cell1).
The MOM h-gates (gh) and the MLP head (fc1/fc2) are computed fully
redundantly on every core so their matmuls fill the AllGather latency
windows and no third collective is needed.

Self-contained: hardcodes all shapes; inputs are the full unsharded
tensors from setup_inputs(); output is the full [B, T, OUT] fp32 array.
"""

import numpy as np
import ml_dtypes

import concourse.bass as bass
import concourse.bacc as bacc
import concourse.mybir as mybir
import concourse.tile as tile
from concourse.bass_utils import run_bass_kernel_spmd

# ---------------------------------------------------------------- constants
LATENT, HIDDEN, OUT, INPUT = 512, 1024, 256, 256
BATCH = 128
NCORE = 8
B = 128            # batch = moving dim
NB = HIDDEN // 128  # 8 feature blocks of the hidden dim
SL = HIDDEN // NCORE  # 128 units per core
GATE_PERM = [0, 1, 3, 2]  # block order [i, f, o, g] from pytorch [i, f, g, o]

BF16 = mybir.dt.bfloat16
F32 = mybir.dt.float32
NP_BF16 = ml_dtypes.bfloat16

FT = mybir.ActivationFunctionType
ALU = mybir.AluOpType


# ---------------------------------------------------------------- host packing
def _lhsT_blocks(w):
    """w: [M, K] fp32 weight (maps K->M). Returns [mt*kt, 128, 128] lhsT blocks,
    m-major: block (m, kk) = w.T[kk*128:(kk+1)*128, m*128:(m+1)*128]."""
    M, K = w.shape
    mt, kt = M // 128, K // 128
    wT = np.ascontiguousarray(w.T)  # [K, M]
    blocks = np.empty((mt * kt, 128, 128), np.float32)
    i = 0
    for m in range(mt):
        for kk in range(kt):
            blocks[i] = wT[kk * 128:(kk + 1) * 128, m * 128:(m + 1) * 128]
            i += 1
    return blocks


def _featT_blocks(x):
    """x: [N, F] activations -> x.T [F, N] -> [F//128, 128, N] blocks."""
    F = x.shape[1]
    xT = np.ascontiguousarray(x.T)  # [F, N]
    return xT.reshape(F // 128, 128, -1)


def pack_inputs(inputs):
    """Returns (in_maps, layout) where in_maps[k] = {'wpack':..., 'bpack':...}."""
    f = lambda n: np.asarray(inputs[n], np.float32)
    W_ih0, W_hh0 = f("W_ih0"), f("W_hh0")
    W_ih1, W_hh1 = f("W_ih1"), f("W_hh1")
    W_lh, W_lc = f("W_lh"), f("W_lc")
    W_fc1, W_fc2 = f("W_fc1"), f("W_fc2")
    Wg_h, Wg_c = f("Wg_h"), f("Wg_c")
    z = f("z")
    start_token = f("start_token")
    b_g0 = f("b_ih0") + f("b_hh0")
    b_g1 = f("b_ih1") + f("b_hh1")

    xT0 = np.broadcast_to(start_token.reshape(1, INPUT), (BATCH, INPUT))

    in_maps = []
    offsets = None
    for k in range(NCORE):
        sl = slice(k * SL, (k + 1) * SL)
        wspec = []  # (name, blocks)
        wspec.append(("wlh", _lhsT_blocks(W_lh)))
        wspec.append(("wlc", _lhsT_blocks(W_lc)))
        wspec.append(("wlc_sl", _lhsT_blocks(W_lc[sl])))
        wspec.append(("whz", _lhsT_blocks(Wg_h[:, HIDDEN:])))
        wspec.append(("wcz", _lhsT_blocks(Wg_c[sl, HIDDEN:])))
        for name, W in (("wih0", W_ih0), ("whh0", W_hh0),
                        ("wih1", W_ih1), ("whh1", W_hh1)):
            gb = [ _lhsT_blocks(W[g * HIDDEN + k * SL : g * HIDDEN + (k + 1) * SL])
                   for g in GATE_PERM ]
            wspec.append((name, np.concatenate(gb, 0)))
        wspec.append(("wgh", _lhsT_blocks(Wg_h[:, :HIDDEN])))
        wspec.append(("wgc", _lhsT_blocks(Wg_c[sl, :HIDDEN])))
        wspec.append(("wfc1", _lhsT_blocks(W_fc1)))
        wspec.append(("wfc2", _lhsT_blocks(W_fc2)))
        wspec.append(("zT", _featT_blocks(z)))
        wspec.append(("xT0", _featT_blocks(xT0)))

        if offsets is None:
            offsets = {}
            o = 0
            for name, blk in wspec:
                offsets[name] = o
                o += blk.shape[0]
            offsets["_nblocks"] = o

        allb = np.concatenate([b for _, b in wspec], 0)  # [NBLK,128,128]
        wpack = np.ascontiguousarray(
            allb.transpose(1, 0, 2).reshape(128, -1)).astype(NP_BF16)

        cols = []  # fp32 bias columns, each [128]
        bcol = {}
        def addcols(name, arr2d):
            bcol[name] = len(cols)
            for c in arr2d:
                cols.append(c)
        addcols("g0b", [b_g0[g * HIDDEN + k * SL: g * HIDDEN + (k + 1) * SL] for g in GATE_PERM])
        addcols("g1b", [b_g1[g * HIDDEN + k * SL: g * HIDDEN + (k + 1) * SL] for g in GATE_PERM])
        addcols("blh", f("b_lh").reshape(NB, 128))
        addcols("blc", f("b_lc").reshape(NB, 128))
        addcols("blc_sl", [f("b_lc")[sl]])
        addcols("bghf", f("bg_h").reshape(NB, 128))
        addcols("bgc_sl", [f("bg_c")[sl]])
        addcols("bfc1", f("b_fc1").reshape(NB, 128))
        addcols("bfc2", f("b_fc2").reshape(2, 128))
        bpack = np.stack(cols, 1).astype(np.float32)  # [128, ncol]

        in_maps.append({"wpack": wpack, "bpack": bpack})

    layout = {"offsets": offsets, "bcol": bcol, "ncol": in_maps[0]["bpack"].shape[1]}
    return in_maps, layout


# ---------------------------------------------------------------- device build
def build_nc(T, layout):
    offs = layout["offsets"]
    bcol = layout["bcol"]
    NBLK = offs["_nblocks"]

    nc = bacc.Bacc("TRN2", target_bir_lowering=False, debug=False,
                   num_devices=NCORE)
    wpack_d = nc.dram_tensor("wpack", [128, NBLK * 128], BF16, kind="ExternalInput")
    bpack_d = nc.dram_tensor("bpack", [128, layout["ncol"]], F32, kind="ExternalInput")
    yout_d = nc.dram_tensor("yout", [T, OUT, B], F32, kind="ExternalOutput")

    RG = [list(range(NCORE))]

    with tile.TileContext(nc) as tc:
        with (
            tc.tile_pool(name="cst", bufs=1) as cst,
            tc.tile_pool(name="st", bufs=2) as st,
            tc.tile_pool(name="psW", bufs=2, space="PSUM") as psW,
            tc.tile_pool(name="psG", bufs=2, space="PSUM") as psG,
            tc.tile_pool(name="psS", bufs=2, space="PSUM") as psS,
            tc.tile_pool(name="dram", bufs=2, space="DRAM") as dram,
        ):
            # ---- resident weights + biases
            wt = cst.tile([128, NBLK * 128], BF16, tag="wt")
            nc.sync.dma_start(wt[:], wpack_d.ap())
            bt = cst.tile([128, layout["ncol"]], F32, tag="bt")
            nc.sync.dma_start(bt[:], bpack_d.ap())

            def wblk(name, i):
                o = offs[name] + i
                return wt[:, o * 128:(o + 1) * 128]

            def bias(name, i=0):
                c = bcol[name] + i
                return bt[:, c:c + 1]

            def mm(ps, pcol, wname, wbase, rhs_tile, rhs_blocks, start, stop):
                """Accumulate sum_kk w[wbase+kk].T @ rhs[:, blk] into ps[:, pcol*B:...]."""
                n = len(rhs_blocks)
                for j, kk in enumerate(rhs_blocks):
                    nc.tensor.matmul(
                        ps[:, pcol * B:(pcol + 1) * B],
                        wblk(wname, wbase + j),
                        rhs_tile[:, kk * B:(kk + 1) * B],
                        start=(start and j == 0),
                        stop=(stop and j == n - 1),
                    )

            # ---- init: h_z, c_z, Hz, Cz
            hz = cst.tile([128, NB * B], BF16, tag="hz")
            cz = cst.tile([128, NB * B], BF16, tag="cz")
            for dst, wname, bname in ((hz, "wlh", "blh"), (cz, "wlc", "blc")):
                ps = psW.tile([128, NB * B], F32, tag="W")
                for m in range(NB):
                    for kk in range(4):
                        nc.tensor.matmul(
                            ps[:, m * B:(m + 1) * B],
                            wblk(wname, m * 4 + kk),
                            wt[:, (offs["zT"] + kk) * 128:(offs["zT"] + kk + 1) * 128],
                            start=(m % 4 == 0 and kk == 0),
                            stop=(m % 4 == 3 and kk == 3))
                for m in range(NB):
                    nc.scalar.activation(dst[:, m * B:(m + 1) * B],
                                         ps[:, m * B:(m + 1) * B],
                                         FT.Tanh, bias=bias(bname, m))
            # c_z slice (fp32)
            cz_sl = cst.tile([128, B], F32, tag="cz_sl")
            ps = psS.tile([128, 2 * B], F32, tag="S")
            for kk in range(4):
                nc.tensor.matmul(ps[:, 0:B], wblk("wlc_sl", kk),
                                 wt[:, (offs["zT"] + kk) * 128:(offs["zT"] + kk + 1) * 128],
                                 start=(kk == 0), stop=(kk == 3))
            nc.scalar.activation(cz_sl[:], ps[:, 0:B], FT.Tanh, bias=bias("blc_sl"))

            # Hz full [128, NB*B] fp32 ; Cz slice [128, B] fp32
            Hz = cst.tile([128, NB * B], F32, tag="Hz")
            ps = psW.tile([128, NB * B], F32, tag="W")
            for m in range(NB):
                mm(ps, m, "whz", m * NB, hz, range(NB), m % 4 == 0, m % 4 == 3)
            for m in range(NB):
                nc.scalar.activation(Hz[:, m * B:(m + 1) * B], ps[:, m * B:(m + 1) * B],
                                     FT.Identity, bias=bias("bghf", m))
            Cz = cst.tile([128, B], F32, tag="Cz")
            ps = psS.tile([128, 2 * B], F32, tag="S")
            mm(ps, 0, "wcz", 0, cz, range(NB), True, True)
            nc.scalar.activation(Cz[:], ps[:, 0:B], FT.Identity, bias=bias("bgc_sl"))

            # ---------------- helpers for the step loop
            def cell(gps, bname, c_prev, payload):
                """LSTM cell elementwise from gate PSUM [128,4B] (blocks i,f,o,g).
                Writes h' (bf16) to payload[:,0:B], c' (bf16) to payload[:,B:2B].
                Returns c_new fp32 tile."""
                sg = st.tile([128, 3 * B], F32, tag="sg")
                for j in range(3):
                    nc.scalar.activation(sg[:, j * B:(j + 1) * B],
                                         gps[:, j * B:(j + 1) * B],
                                         FT.Sigmoid, bias=bias(bname, j))
                tg = st.tile([128, B], F32, tag="tg")
                nc.scalar.activation(tg[:], gps[:, 3 * B:4 * B], FT.Tanh,
                                     bias=bias(bname, 3))
                t1 = st.tile([128, B], F32, tag="t1")
                nc.vector.tensor_tensor(t1[:], sg[:, B:2 * B], c_prev[:], ALU.mult)
                t2 = st.tile([128, B], F32, tag="t2")
                nc.vector.tensor_tensor(t2[:], sg[:, 0:B], tg[:], ALU.mult)
                cn = st.tile([128, B], F32, tag="cn")
                nc.vector.tensor_tensor(cn[:], t1[:], t2[:], ALU.add)
                tc_ = st.tile([128, B], F32, tag="tc")
                nc.scalar.activation(tc_[:], cn[:], FT.Tanh)
                nc.vector.tensor_tensor(payload[:, 0:B], sg[:, 2 * B:3 * B], tc_[:], ALU.mult)
                nc.vector.tensor_copy(payload[:, B:2 * B], cn[:])
                return cn

            def allgather(payload, tag):
                """payload [128,2B] bf16 -> gathered (hX [128,NB*B], cX [128,NB*B])."""
                ain = dram.tile([128, 2 * B], BF16, tag=f"{tag}_in")
                nc.sync.dma_start(ain[:], payload[:])
                aout = dram.tile([NCORE * 128, 2 * B], BF16, tag=f"{tag}_out")
                nc.gpsimd.collective_compute(
                    "AllGather", ALU.bypass, replica_groups=RG,
                    ins=[ain.opt()], outs=[aout.opt()])
                hp = st.tile([128, NB * B], BF16, tag=f"{tag}_h")
                cp = st.tile([128, NB * B], BF16, tag=f"{tag}_c")
                nc.sync.dma_start(
                    hp[:], aout[:, 0:B].rearrange("(j p) b -> p j b", p=128))
                nc.sync.dma_start(
                    cp[:], aout[:, B:2 * B].rearrange("(j p) b -> p j b", p=128))
                return hp, cp

            def mom_h(hp, hf_tag):
                """Full redundant MOM h-gate: returns new full h'' bf16 [128,NB*B]."""
                ps = psW.tile([128, NB * B], F32, tag="W")
                for m in range(NB):
                    mm(ps, m, "wgh", m * NB, hp, range(NB), m % 4 == 0, m % 4 == 3)
                s = st.tile([128, NB * B], F32, tag="mhs")
                nc.vector.tensor_tensor(s[:], ps[:], Hz[:], ALU.add)
                gh = st.tile([128, NB * B], BF16, tag="mhg")
                nc.scalar.activation(gh[:], s[:], FT.Sigmoid)
                d = st.tile([128, NB * B], BF16, tag="mhd")
                nc.vector.tensor_tensor(d[:], hp[:], hz[:], ALU.subtract)
                m_ = st.tile([128, NB * B], BF16, tag="mhm")
                nc.vector.tensor_tensor(m_[:], gh[:], d[:], ALU.mult)
                hf = st.tile([128, NB * B], BF16, tag=hf_tag)
                nc.vector.tensor_tensor(hf[:], hz[:], m_[:], ALU.add)
                return hf

            def mom_c(cp, cn, c_tag):
                """Sliced MOM c-gate: returns new c state fp32 [128,B]."""
                ps = psS.tile([128, 2 * B], F32, tag="S")
                mm(ps, 0, "wgc", 0, cp, range(NB), True, True)
                sc = st.tile([128, B], F32, tag="mcs")
                nc.vector.tensor_tensor(sc[:], ps[:, 0:B], Cz[:], ALU.add)
                gc = st.tile([128, B], F32, tag="mcg")
                nc.scalar.activation(gc[:], sc[:], FT.Sigmoid)
                d = st.tile([128, B], F32, tag="mcd")
                nc.vector.tensor_tensor(d[:], cn[:], cz_sl[:], ALU.subtract)
                m_ = st.tile([128, B], F32, tag="mcm")
                nc.vector.tensor_tensor(m_[:], gc[:], d[:], ALU.mult)
                c = st.tile([128, B], F32, tag=c_tag)
                nc.vector.tensor_tensor(c[:], cz_sl[:], m_[:], ALU.add)
                return c

            # ---------------- prologue: step 0 front half
            # G0(0): h-part (rhs hz), x-part (rhs xT0 in wt)
            g0 = psG.tile([128, 4 * B], F32, tag="G")
            for g in range(4):
                mm(g0, g, "whh0", g * NB, hz, range(NB), g == 0, False)
            xT0blocks = wt[:, offs["xT0"] * 128:(offs["xT0"] + 2) * 128]
            for g in range(4):
                mm(g0, g, "wih0", g * 2, xT0blocks, range(2), False, g == 3)
            pA = st.tile([128, 2 * B], BF16, tag="pA")
            c0n = cell(g0, "g0b", cz_sl, pA)
            h0p, c0p = allgather(pA, "agA")
            # G1h(0)
            g1 = psG.tile([128, 4 * B], F32, tag="G")
            for g in range(4):
                mm(g1, g, "whh1", g * NB, hz, range(NB), g == 0, False)

            c0, c1 = cz_sl, cz_sl
            h0f = h1f = None

            # ---------------- main loop
            for t in range(T):
                last = (t == T - 1)
                # G1x(t)
                for g in range(4):
                    mm(g1, g, "wih1", g * NB, h0p, range(NB), False, g == 3)
                pB = st.tile([128, 2 * B], BF16, tag="pB")
                c1n = cell(g1, "g1b", c1, pB)
                h1p, c1p = allgather(pB, "agB")

                # gh0(t) -> h0f ; gc0(t) -> c0   (fills AG_B window)
                if not last:
                    h0f = mom_h(h0p, "h0f")
                    c0 = mom_c(c0p, c0n, "c0")

                # fc1(t): full, rhs h1p
                ps = psW.tile([128, NB * B], F32, tag="W")
                for m in range(NB):
                    mm(ps, m, "wfc1", m * NB, h1p, range(NB), m % 4 == 0, m % 4 == 3)
                relu = st.tile([128, NB * B], BF16, tag="relu")
                for m in range(NB):
                    nc.scalar.activation(relu[:, m * B:(m + 1) * B],
                                         ps[:, m * B:(m + 1) * B],
                                         FT.Relu, bias=bias("bfc1", m))
                # fc2(t)
                yps = psS.tile([128, 2 * B], F32, tag="S")
                for m in range(2):
                    mm(yps, m, "wfc2", m * NB, relu, range(NB), m == 0, m == 1)
                y32 = st.tile([128, 2 * B], F32, tag="y32")
                for m in range(2):
                    nc.scalar.activation(y32[:, m * B:(m + 1) * B],
                                         yps[:, m * B:(m + 1) * B],
                                         FT.Identity, bias=bias("bfc2", m))
                nc.sync.dma_start(
                    yout_d[t].rearrange("(m p) b -> p m b", p=128), y32[:])

                if last:
                    break

                xT = st.tile([128, 2 * B], BF16, tag="xT")
                nc.vector.tensor_copy(xT[:], y32[:])

                # G0(t+1)
                g0 = psG.tile([128, 4 * B], F32, tag="G")
                for g in range(4):
                    mm(g0, g, "whh0", g * NB, h0f, range(NB), g == 0, False)
                for g in range(4):
                    mm(g0, g, "wih0", g * 2, xT, range(2), False, g == 3)
                pA = st.tile([128, 2 * B], BF16, tag="pA")
                c0n = cell(g0, "g0b", c0, pA)
                h0p, c0p = allgather(pA, "agA")

                # gh1(t) -> h1f ; gc1(t) -> c1   (fills AG_A window)
                h1f = mom_h(h1p, "h1f")
                c1 = mom_c(c1p, c1n, "c1")

                # G1h(t+1)
                g1 = psG.tile([128, 4 * B], F32, tag="G")
                for g in range(4):
                    mm(g1, g, "whh1", g * NB, h1f, range(NB), g == 0, False)

    nc.compile()
    return nc


# ---------------------------------------------------------------- entry point
_CACHE = {}


def kernel(**inputs):
    T = int(inputs["seq_length"])
    in_maps, layout = pack_inputs(inputs)
    key = T
    if key not in _CACHE:
        _CACHE[key] = build_nc(T, layout)
    nc = _CACHE[key]
    res = run_bass_kernel_spmd(nc, in_maps, list(range(NCORE)))
    yout = res.results[0]["yout"]  # [T, OUT, B]
    return np.ascontiguousarray(yout.transpose(2, 0, 1)).astype(np.float32)


if __name__ == "__main__":
    import reference as ref
    inputs = ref.setup_inputs()
    inputs = dict(inputs)
    inputs["seq_length"] = 4
    out = kernel(**inputs)
    print("out shape:", out.shape, out.dtype)
